# revision 56
# baseline (speedup 1.0000x reference)
"""Trainium2 Bass kernel for causal multi-head attention (GPT-style block).

Reference computation (per batch b):
    qkv = x @ w_attn + b_attn ; q,k,v = split(qkv)
    per head: S = q k^T / sqrt(64); causal mask; P = softmax(S); a = P v
    out = concat_heads(a) @ w_proj + b_proj

Shapes: x (2, 2048, 1024), 16 heads, head_dim 64.

Sharding: 8 cores = 2 batches x 4 head-groups (4 heads each).  Tensor
parallel over heads: each core computes the QKV projection for its 4 heads
(column slice of w_attn), full causal attention for those heads, and its
partial output projection (row slice of w_proj).  Host sums the 4
head-group partials per batch (bf16 partials, f32 host sum) and adds
b_proj.

On-chip layouts (per core, T=2048, CW=256=4*64):
    xT      [1024, T]   x transposed (host-prepped bf16), streamed per chunk
    Q^T,K^T [CW, T]     projections with head-channel on partitions (bf16)
    V_aug   [T, 4*65]   V natural layout + ones column per head (the ones
                        column makes the PV matmul also produce the softmax
                        denominator as output row 64)
    S^T     [k, q]      scores transposed: PSUM [128, 512] per (k-tile,
                        q-chunk); P^T = exp(S^T/8) directly feeds PV as the
                        moving operand - no transposes in the hot loop.

All matmuls are bf16.  The two heads of a pair are row-tiled (base
partitions 0/64) so their S matmuls run concurrently in the PE array.
Causal masking: off-band blocks are skipped; band blocks are exp'd only on
their live columns, the above-diagonal triangle is zeroed with
gpsimd.affine_select, and the PV matmul uses width-sliced operands so the
fully-masked columns are never touched (no memsets).

The attention inner loop is the only dependency-limited stretch (S ->
exp -> mask -> PV).  To keep the PE dense there (TRN2's HAM re-throttles
the PE clock to 1.2 GHz whenever an activity window contains idle), all
other matmul work - the NEXT chunk's QKV projection groups, V
transposes, and the PREVIOUS chunk's output projection - is queued as
"filler" ops and drained a few per attention step between the S and PV
matmuls.  PSUM banks: 0-3 S-blocks (lookahead 2 x 2 heads), 4/5 PV
accumulators for the head pair, 6/7 toggling filler accumulators.

Perf notes (v2):
  - gpsimd `attn` ucode library preloaded at kernel start (the implicit
    mid-kernel LOAD_LIB for partition_broadcast stalled the gpsimd queue
    ~7.6us, and the next chunk's mask ops sat behind it).
  - pair finalize quick-evacs the PV accumulator to SBUF with two DVE
    copies; reciprocal/broadcast/normalize run lazily so the next chunk's
    PV (same PSUM banks) isn't WAR-blocked on that chain.
  - wqkv/x DMAs are j-sliced so the first QKV matmul starts ~1.5us in
    (was 19.6us), with identity warm-up matmuls covering the DMA wait to
    flip the HAM clock gate early.
  - output partials stored/DMA'd as bf16 (halves out traffic, 2x evac).
"""

import sys

sys.path.insert(0, "/opt/trn_rl_repo")

import numpy as np
import ml_dtypes

import concourse.bacc as bacc
import concourse.mybir as mybir
import concourse.tile as tile
from concourse import library_config
from concourse.bass_utils import run_bass_kernel_spmd

F32 = mybir.dt.float32
F32R = mybir.dt.float32r
BF16 = mybir.dt.bfloat16
NP_BF16 = np.dtype(ml_dtypes.bfloat16)

B = 2
T = 2048
NX = 1024
H = 16
HD = 64
NCORES = 8
NHG = 4          # head groups (cores per batch)
NH = 4           # heads per core
CW = NH * HD     # 256 channel width per core
QC = 512         # q-chunk (moving dim)
NQC = T // QC    # 4
KT = 128         # k-tile
VW = HD + 1      # 65: V columns + ones column
NWARM = 16       # identity warm-up matmuls at t0


def _build():
    nc = bacc.Bacc("TRN2", target_bir_lowering=False, debug=False,
                   num_devices=NCORES)
    xT_d = nc.dram_tensor("xT", [NX, T], BF16, kind="ExternalInput")
    wqkv_d = nc.dram_tensor("wqkv", [128, 8 * 3 * CW], BF16, kind="ExternalInput")
    bias_d = nc.dram_tensor("bias", [128, 6], F32, kind="ExternalInput")
    wp_d = nc.dram_tensor("wp", [128, 2 * NX], BF16, kind="ExternalInput")
    ident_d = nc.dram_tensor("ident", [128, 128], F32R, kind="ExternalInput")
    onesf_d = nc.dram_tensor("onesf", [1, 64], F32R, kind="ExternalInput")
    vones_d = nc.dram_tensor("vones", [128, 16 * NH], BF16, kind="ExternalInput")
    out_d = nc.dram_tensor("out_p", [T, NX], BF16, kind="ExternalOutput")

    Exp = mybir.ActivationFunctionType.Exp

    with tile.TileContext(nc) as tc:
        with (
            tc.tile_pool(name="pers", bufs=1) as pers,
            tc.tile_pool(name="xin", bufs=4) as xin,
            tc.tile_pool(name="ps", bufs=1, space="PSUM") as psum,
            tc.tile_pool(name="ptp", bufs=10) as ptp,
            tc.tile_pool(name="stg", bufs=4) as stg,
            tc.tile_pool(name="op", bufs=6) as op,
            tc.tile_pool(name="rp", bufs=6) as rp,
        ):
            def bank(i, shape, dtype=F32):
                return psum.tile(shape, dtype, tag=f"bank{i}", bufs=1,
                                 name=f"bank{i}")

            # ---- persistent tiles / startup ----
            # ident first (tiny) - feeds the HAM warm-up matmuls below.
            ident = pers.tile([128, 128], F32R, tag="ident")
            nc.sync.dma_start(ident[:], ident_d.ap())
            # x chunk 0 in two j-halves on the gpsimd (SWDGE) queue so the
            # first QKV groups have data early; wqkv j-sliced on sync
            # (HWDGE) in need-order.
            def load_x(qq):
                xt = xin.tile([128, 8, QC], BF16, tag="xt")
                src = xT_d.ap().rearrange("(j p) t -> p j t",
                                          p=128)[:, :, qq * QC:(qq + 1) * QC]
                nc.gpsimd.dma_start(xt[:], src)
                return xt

            # chunk-0 x rides the sync queue in two j-halves around the
            # wqkv j-slices (the gpsimd SWDGE queue takes ~7us to deliver
            # its first bytes, which stalled the first QKV group to 9.7us)
            xt0 = xin.tile([128, 8, QC], BF16, tag="xt")
            xt0_src = xT_d.ap().rearrange("(j p) t -> p j t",
                                          p=128)[:, :, 0:QC]
            # preload the gpsimd ucode library that partition_broadcast
            # needs; otherwise the compiler inserts the load mid-kernel
            # and the mask ops queue behind a ~7us library fetch.
            nc.gpsimd.load_library(library_config.attn)
            wqkv = pers.tile([128, 8, 3 * CW], BF16, tag="wqkv")
            wqkv_src = wqkv_d.ap().rearrange("p (j c) -> p j c", j=8)
            nc.sync.dma_start(xt0[:, 0:4, :], xt0_src[:, 0:4, :])
            for j in range(8):
                nc.sync.dma_start(wqkv[:, j, :], wqkv_src[:, j, :])
                if j == 2:
                    bias = pers.tile([128, 6], F32, tag="bias")
                    nc.sync.dma_start(bias[:], bias_d.ap())
                if j == 3:
                    nc.sync.dma_start(xt0[:, 4:8, :], xt0_src[:, 4:8, :])
            vaug = pers.tile([128, T // KT, NH * VW], BF16, tag="vaug")
            vones_sb = pers.tile([128, 16 * NH], BF16, tag="vones")
            nc.sync.dma_start(vones_sb[:], vones_d.ap())
            onesf = pers.tile([1, 64], F32R, tag="onesf")
            nc.sync.dma_start(onesf[:], onesf_d.ap())
            wp = pers.tile([128, 2, NX], BF16, tag="wp")
            nc.sync.dma_start(wp[:], wp_d.ap().rearrange("p (c n) -> p c n", c=2))

            # HAM warm-up: keep the PE busy while the first weight/x DMA
            # slices land so the clock gate opens before real work.
            warm = bank(6, [128, QC])
            for i in range(NWARM):
                nc.tensor.matmul(warm[:, 0:128], ident[:], ident[:],
                                 start=True, stop=True)
            # preload the ACT exp table set (~2.7us) during the DMA wait
            # instead of at the first real exp inside chunk-0 attention
            dumm = rp.tile([1, QC], F32, tag="dn")
            nc.scalar.activation(dumm[0:1, 0:6], bias[0:1, :], Exp,
                                 scale=0.125)

            # ones columns via DVE fill (element-exact; a sub-512B strided
            # DMA would read-modify-write and race the V data copies)
            nc.vector.tensor_copy(
                vaug[:].rearrange("p t (h w) -> p t h w", h=NH)[:, :, :, HD:HD + 1],
                vones_sb[:].rearrange("p (t h w) -> p t h w", t=16, h=NH),
            )

            QT = [pers.tile([128, T], BF16, tag=f"qt{i}", name=f"qt{i}")
                  for i in range(2)]
            KTs = [pers.tile([128, T], BF16, tag=f"kt{i}", name=f"kt{i}")
                   for i in range(2)]
            anorm = [pers.tile([128, T], BF16, tag=f"an{i}", name=f"an{i}")
                     for i in range(2)]

            fl_state = {"toggle": 0, "bank": None}

            def fl_bank(shape, dtype=F32):
                fl_state["toggle"] ^= 1
                fl_state["bank"] = bank(6 + fl_state["toggle"], shape, dtype)
                return fl_state["bank"]

            def v_transpose_ops(qq, vstages):
                """Filler ops: PE-transpose V^T chunk -> V natural in vaug.
                blk-major (c2 pairs per block) so early k-tiles' V lands
                first when drained in the consuming chunk."""
                ops = []
                for blk in range(4):
                    for c2 in range(2):
                        def f(c2=c2, blk=blk):
                            vs = vstages[c2]
                            pt_ps = fl_bank([128, 128], F32R)
                            nc.tensor.transpose(
                                pt_ps[:], vs[:, blk * 128:(blk + 1) * 128],
                                ident[:])
                            tt = qq * 4 + blk
                            dst = vaug[:, tt,
                                       c2 * 2 * VW:c2 * 2 * VW + 2 * VW]
                            dst = dst.rearrange("p (h w) -> p h w",
                                                h=2)[:, :, 0:HD]
                            src = pt_ps[:].rearrange("p (h w) -> p h w", h=2)
                            nc.vector.tensor_copy(dst, src)
                        ops.append(f)
                return ops

            def qkv_ops(qq, xt):
                """Filler ops for QKV projection of chunk qq, split into
                (qk_ops, v_ops, trans_ops) so the scheduler can balance
                engine load across chunks.  6 groups (q/k/v x c2-half) of 8
                accumulating matmuls into a toggling filler bank; evac on
                DVE (with bias)."""
                cs = slice(qq * QC, (qq + 1) * QC)
                vstages = [None, None]
                qk_ops = []
                v_ops = []
                for off, kind in ((0, "q"), (CW, "k"), (2 * CW, "v")):
                    for c2 in range(2):
                        for j in range(8):
                            def f(off=off, kind=kind, c2=c2, j=j):
                                if j == 0:
                                    fl_bank([128, QC])
                                g = fl_state["bank"]
                                lhsT = wqkv[:, j,
                                            off + c2 * 128:off + (c2 + 1) * 128]
                                nc.tensor.matmul(g[:], lhsT, xt[:, j, :],
                                                 start=(j == 0), stop=(j == 7))
                                if j == 7:
                                    bcol = {"q": 0, "k": 2, "v": 4}[kind] + c2
                                    bap = bias[:, bcol:bcol + 1]
                                    if kind == "q":
                                        nc.vector.tensor_scalar_add(
                                            QT[c2][:, cs], g[:], bap)
                                    elif kind == "k":
                                        nc.vector.tensor_scalar_add(
                                            KTs[c2][:, cs], g[:], bap)
                                    else:
                                        vs = stg.tile([128, QC], F32R,
                                                      tag="vstage")
                                        nc.vector.tensor_scalar_add(
                                            vs[:], g[:], bap)
                                        vstages[c2] = vs
                            (qk_ops if kind != "v" else v_ops).append(f)
                return qk_ops, v_ops, v_transpose_ops(qq, vstages)

            def qkv0_dense(xt):
                """Chunk-0 QKV, j-outer across all 6 groups in 6 PSUM
                accumulators so the first wqkv/x j-slices feed 6 matmuls
                each and the PE never waits for a later DMA slice."""
                vstages = [None, None]
                groups = [(off, kind, c2)
                          for off, kind in ((0, "q"), (CW, "k"), (2 * CW, "v"))
                          for c2 in range(2)]
                banks = [bank(gi, [128, QC])[:] for gi in range(6)]
                for j in range(8):
                    for gi, (off, kind, c2) in enumerate(groups):
                        lhsT = wqkv[:, j, off + c2 * 128:off + (c2 + 1) * 128]
                        nc.tensor.matmul(banks[gi], lhsT, xt[:, j, :],
                                         start=(j == 0), stop=(j == 7))
                for gi, (off, kind, c2) in enumerate(groups):
                    bcol = {"q": 0, "k": 2, "v": 4}[kind] + c2
                    bap = bias[:, bcol:bcol + 1]
                    if kind == "q":
                        nc.vector.tensor_scalar_add(QT[c2][:, 0:QC],
                                                    banks[gi], bap)
                    elif kind == "k":
                        nc.vector.tensor_scalar_add(KTs[c2][:, 0:QC],
                                                    banks[gi], bap)
                    else:
                        vs = stg.tile([128, QC], F32R, tag="vstage")
                        nc.vector.tensor_scalar_add(vs[:], banks[gi], bap)
                        vstages[c2] = vs
                for f in v_transpose_ops(0, vstages):
                    f()

            # out-DMAs ride the sync HWDGE queue; only the final chunk's
            # (attention done, exp pacing no longer matters) alternate
            # onto the Activation HWDGE queue to halve the tail drain.
            # Putting mid-kernel triggers on the ACT queue perturbed the
            # exp stream enough to flip the HAM into its slow mode.
            dma_rr = {"i": 0}

            def out_dma(dst, src, split=False):
                # final chunk only: alternate onto the Activation HWDGE
                # queue (attention is over, so ACT-queue jitter is safe)
                dma_rr["i"] += 1
                if split and dma_rr["i"] % 2 == 0:
                    nc.scalar.dma_start(dst, src)
                else:
                    nc.sync.dma_start(dst, src)

            def cproj_ops(qq, act_ok=False):
                """Filler ops: output projection for t-rows of chunk qq.
                act_ok: allow ScalarE evacs (final dense chunk only - during
                interleaved chunks the ScalarE paces the attention exps)."""
                ops = []
                for i in range(4):
                    tt = qq * 4 + i
                    for nxc in range(2):
                        for c2 in range(2):
                            def f(tt=tt, nxc=nxc, c2=c2, i=i):
                                if c2 == 0:
                                    fl_bank([128, QC])
                                po = fl_state["bank"]
                                nc.tensor.matmul(
                                    po[:],
                                    anorm[c2][:, tt * 128:(tt + 1) * 128],
                                    wp[:, c2, nxc * QC:(nxc + 1) * QC],
                                    start=(c2 == 0), stop=(c2 == 1))
                                if c2 == 1:
                                    ot = op.tile([128, QC], BF16, tag="ot")
                                    if not act_ok:
                                        nc.vector.tensor_copy(ot[:], po[:])
                                    else:
                                        # final dense chunk: split evacs
                                        # across ACT and DVE so both
                                        # drain in parallel
                                        if (i * 2 + nxc) % 3 == 0:
                                            nc.scalar.copy(ot[:], po[:])
                                        else:
                                            nc.vector.tensor_copy(ot[:], po[:])
                                    out_dma(
                                        out_d.ap()[tt * 128:(tt + 1) * 128,
                                                   nxc * QC:(nxc + 1) * QC],
                                        ot[:], split=act_ok)
                            ops.append(f)
                return ops

            def attention_pair(hp, qq, fillers, steps_left, fast_fin=False):
                """Heads (2hp, 2hp+1) for q-chunk qq, S/PV interleaved with
                filler drain (spread adaptively over remaining steps)."""
                c2 = hp
                nk = 4 * qq + 4
                qs = slice(qq * QC, (qq + 1) * QC)
                pa = [bank(4, [VW, QC]), bank(5, [VW, QC])]
                pts = {}
                LA = 2

                def s_block(kk, hh):
                    ps_s = bank((2 * kk + hh) % 4, [128, QC])
                    rows = slice(64 * hh, 64 * hh + 64)
                    lhsT = KTs[c2][rows, kk * KT:(kk + 1) * KT]
                    rhs = QT[c2][rows, qs]
                    nc.tensor.matmul(ps_s[:], lhsT, rhs, start=True, stop=True)
                    pt = ptp.tile([128, QC], BF16, tag="pt")
                    if kk >= 4 * qq:
                        # band block: columns < 128j are fully masked - the
                        # exp and the PV matmul skip them entirely (sliced
                        # width); affine_select zeroes the above-diagonal
                        # triangle of the first live 128 columns.
                        j = kk - 4 * qq
                        nc.scalar.activation(pt[:, 128 * j:QC],
                                             ps_s[:, 128 * j:QC], Exp,
                                             scale=0.125)
                        nc.gpsimd.affine_select(
                            pt[:, 128 * j:128 * (j + 1)],
                            pt[:, 128 * j:128 * (j + 1)],
                            pattern=[[1, 128]],
                            compare_op=mybir.AluOpType.is_ge, fill=0.0,
                            base=0, channel_multiplier=-1)
                    else:
                        nc.scalar.activation(pt[:], ps_s[:], Exp, scale=0.125)
                    pts[(kk, hh)] = pt

                def pv_block(kk, hh):
                    h = 2 * hp + hh
                    lhsT = vaug[:, kk, h * VW:(h + 1) * VW]
                    pt = pts.pop((kk, hh))
                    if kk >= 4 * qq:
                        # band block: only live columns participate
                        j = kk - 4 * qq
                        nc.tensor.matmul(pa[hh][:, 128 * j:],
                                         lhsT, pt[:, 128 * j:],
                                         start=(kk == 0),
                                         stop=(kk == nk - 1))
                    else:
                        nc.tensor.matmul(pa[hh][:], lhsT, pt[:],
                                         start=(kk == 0), stop=(kk == nk - 1))

                for kk in range(min(LA, nk)):
                    s_block(kk, 0)
                    s_block(kk, 1)
                for kk in range(nk):
                    if kk + LA < nk:
                        s_block(kk + LA, 0)
                        s_block(kk + LA, 1)
                    # floor-based pacing: drains evenly with the remainder
                    # landing in the LAST steps, so the PE stays fed at the
                    # end of the chunk (ceil pacing exhausted fillers a few
                    # steps early and the HAM re-throttled the clock).
                    nf = len(fillers)
                    sl = max(1, steps_left[0])
                    n = max(nf // sl, 1 if nf else 0)
                    steps_left[0] -= 1
                    for _ in range(n):
                        if fillers:
                            fillers.pop(0)()
                    pv_block(kk, 0)
                    pv_block(kk, 1)

                # pair finalize: quick-evac the PV accumulators to SBUF
                # (frees PSUM banks 4/5 for the next pair after two DVE
                # copies); reciprocal/broadcast/normalize run lazily off
                # the critical path.  For the last pair (fast_fin) the
                # partition-broadcast runs as a tiny PE matmul into a free
                # S bank instead of the ~1.2us gpsimd op, since the final
                # c_proj is serially blocked on this chain.
                for hh in range(2):
                    rows = slice(64 * hh, 64 * hh + 64)
                    ast = rp.tile([64, QC], BF16, tag="ast")
                    nc.vector.tensor_copy(ast[:], pa[hh][0:HD, :])
                    dn = rp.tile([1, QC], F32, tag="dn")
                    nc.vector.tensor_copy(dn[:], pa[hh][HD:HD + 1, :])
                    recip = rp.tile([1, QC], F32, tag="recip")
                    nc.vector.reciprocal_approx_fast(recip[:], dn[:])
                    if fast_fin:
                        recipr = rp.tile([1, QC], F32R, tag="recipr")
                        nc.vector.tensor_copy(recipr[:], recip[:])
                        rps = bank(2 * hh, [128, QC])[0:64, :]
                        nc.tensor.matmul(rps, onesf[:], recipr[:],
                                         start=True, stop=True)
                        nc.vector.tensor_mul(anorm[c2][rows, qs],
                                             ast[:], rps)
                    else:
                        rbc = rp.tile([64, QC], F32, tag="rbc")
                        nc.gpsimd.partition_broadcast(rbc[:], recip[:])
                        nc.vector.tensor_mul(anorm[c2][rows, qs],
                                             ast[:], rbc[:])

            # ---- main pipeline over q-chunks ----
            # chunk 0 QKV runs dense up front (j-interleaved).  Filler
            # assignment balances PE work against the ACT exp pacing per
            # chunk: early chunks are PE-rich (QKV projections), so the
            # c_proj work all drains in chunk 3 whose attention stretch is
            # otherwise ACT-bound (PE starvation there made the HAM clock
            # gate re-throttle the PE to 1.2 GHz for ~40us in v2).
            #   chunk 0: qk(1) + v(1) + trans(1)     [56 ops]
            #   chunk 1: qk(2)                       [32]
            #   chunk 2: v(2) + trans(2) + qk(3) + v(3)  [72]
            #   chunk 3: trans(3) + cproj(0,1,2)     [56]
            qkv0_dense(xt0)
            qk1, v1, tr1 = qkv_ops(1, load_x(1))
            qk2, v2, tr2 = qkv_ops(2, load_x(2))
            qk3, v3, tr3 = qkv_ops(3, load_x(3))
            chunk_fillers = [
                qk1 + v1 + tr1,
                qk2,
                v2 + tr2 + qk3 + v3,
                tr3 + cproj_ops(0) + cproj_ops(1) + cproj_ops(2),
            ]
            for qq in range(NQC):
                fillers = chunk_fillers[qq]
                steps_left = [2 * (4 * qq + 4)]
                for hp in range(2):
                    attention_pair(hp, qq, fillers, steps_left,
                                   fast_fin=(qq == NQC - 1 and hp == 1))
                while fillers:
                    fillers.pop(0)()
            for f in cproj_ops(NQC - 1, act_ok=True):
                f()

    nc.compile()
    return nc


_CACHE = {}


def _get_nc():
    if "nc" not in _CACHE:
        _CACHE["nc"] = _build()
    return _CACHE["nc"]


def kernel(x, w_attn, b_attn, w_proj, b_proj):
    x = np.asarray(x, dtype=np.float32)
    w_attn = np.asarray(w_attn, dtype=np.float32)
    b_attn = np.asarray(b_attn, dtype=np.float32)
    w_proj = np.asarray(w_proj, dtype=np.float32)
    b_proj = np.asarray(b_proj, dtype=np.float32)

    ident = np.eye(128, dtype=np.float32)
    vones = np.ones((128, 64), dtype=NP_BF16)
    in_maps = []
    for core in range(NCORES):
        b, hg = divmod(core, NHG)
        cols = slice(hg * CW, (hg + 1) * CW)
        bias = np.empty((128, 6), dtype=np.float32)
        for qkv_i in range(3):
            bseg = b_attn[qkv_i * NX:][cols]
            bias[:, 2 * qkv_i] = bseg[:128]
            bias[:, 2 * qkv_i + 1] = bseg[128:]
        in_maps.append({
            "xT": np.ascontiguousarray(x[b].T).astype(NP_BF16),
            "onesf": np.ones((1, 64), dtype=np.float32),
            "wqkv": np.concatenate(
                [w_attn[:, cols], w_attn[:, NX:][:, cols],
                 w_attn[:, 2 * NX:][:, cols]], axis=1).astype(NP_BF16)
                .reshape(8, 128, 3 * CW).transpose(1, 0, 2).reshape(128, -1)
                .copy(),
            "bias": bias,
            "wp": np.ascontiguousarray(w_proj[cols, :]).astype(NP_BF16)
                .reshape(2, 128, NX).transpose(1, 0, 2).reshape(128, -1).copy(),
            "ident": ident,
            "vones": vones,
        })

    nc = _get_nc()
    res = run_bass_kernel_spmd(nc, in_maps, core_ids=list(range(NCORES)))
    _CACHE["last_res"] = res
    out = np.empty((B, T, NX), dtype=np.float32)
    for b in range(B):
        acc = res.results[b * NHG]["out_p"].astype(np.float32)
        for hg in range(1, NHG):
            acc = acc + res.results[b * NHG + hg]["out_p"].astype(np.float32)
        out[b] = acc + b_proj
    return out
